# revision 4
# baseline (speedup 1.0000x reference)
"""nn_AttentionConv_32487132627486 — Bass/Tile kernel for 8 Trainium2 NeuronCores.

Sharding: data-parallel over batch (B=32 -> 4 batches per core). Each core
processes its 4 batches as two "pair tiles" of 2 batches stacked on the
128 SBUF partitions: partition p = (b2, c) with b2 in {0,1}, c in 0..63.

Per pair tile:
  - PE projections q/k/v via block-diagonal weights (both batches at once).
  - For each of the 49 window shifts n=(ki,kj):
      DVE:  tmp_n = (k_shifted + rel_col[:,n]) * q        (fused STT, fp32)
      PE :  scores_n[(b2,c),ij] = sum_{c' in group} tmp_n  (matmul with group
            mask M2 -> group-reduced scores, pre-broadcast over the 8 channels)
      ACT:  e_n = exp(scores_n - 15)  (PSUM -> SBUF bf16; max-free softmax:
            logits are O(+-40) so exp stays in fp32/bf16 range; the -15 shift
            cancels in the normalization)
      DVE:  u_n = e_n * v_shifted      (bf16)
      PE :  acc_u += u_n ; acc_e += e_n  (identity matmul accumulate in PSUM)
  - out = acc_u * (1/acc_e) * mask     (DVE; reciprocal_approx_fast)

v is stored twice in bf16 (normal and shifted one column) so every shifted
view is 4-byte aligned, keeping the DVE tensor_tensor ops in 2x bf16 mode.
"""

import numpy as np
import ml_dtypes

# ---- static config (hardcoded per spec) ----
B, CIN, H, W = 32, 64, 32, 32
CO, KK, G, PAD = 64, 7, 8, 3
R_RAMP = 3.0
MAXSZ = W // 2          # 16
CPG = CO // G           # 8
HP = H + 2 * PAD        # 38
NSH = KK * KK           # 49
NCORES = 8
BLOC = B // NCORES      # 4 batches per core
NT = BLOC // 2          # 2 pair-tiles per core
EXP_BIAS = -15.0

_CACHE = {}


def _build_nc():
    """Build and compile the per-core Bass module (same NEFF on all 8 cores)."""
    from concourse.bacc import Bacc
    import concourse.mybir as mybir
    import concourse.tile as tile

    f32 = mybir.dt.float32
    bf16 = mybir.dt.bfloat16
    AL = mybir.AluOpType
    AF = mybir.ActivationFunctionType

    nc = Bacc("TRN2")
    x_d = nc.dram_tensor("x", [BLOC, CIN, H, W], f32, kind="ExternalInput")
    wq_d = nc.dram_tensor("w2q", [128, 128], f32, kind="ExternalInput")
    wk_d = nc.dram_tensor("w2k", [128, 128], f32, kind="ExternalInput")
    wv_d = nc.dram_tensor("w2v", [128, 128], f32, kind="ExternalInput")
    m2_d = nc.dram_tensor("m2", [128, 128], f32, kind="ExternalInput")
    id_d = nc.dram_tensor("idb", [128, 128], bf16, kind="ExternalInput")
    rel_d = nc.dram_tensor("relc", [128, NSH], f32, kind="ExternalInput")
    msk_d = nc.dram_tensor("maskx", [128, H, W], f32, kind="ExternalInput")
    y_d = nc.dram_tensor("y", [BLOC, CO, H, W], f32, kind="ExternalOutput")

    with tile.TileContext(nc) as tc:
        with (
            tc.tile_pool(name="const", bufs=1) as cp,
            tc.tile_pool(name="data", bufs=2) as dp,
            tc.tile_pool(name="work", bufs=3) as wp,
            tc.tile_pool(name="mm", bufs=2, space="PSUM") as mp,
            tc.tile_pool(name="accp", bufs=1, space="PSUM") as accp,
        ):
            w2q = cp.tile([128, 128], f32)
            w2k = cp.tile([128, 128], f32)
            w2v = cp.tile([128, 128], f32)
            m2 = cp.tile([128, 128], f32)
            idb = cp.tile([128, 128], bf16)
            relc = cp.tile([128, NSH], f32)
            maskx = cp.tile([128, H, W], f32)
            ebias = cp.tile([128, 1], f32)
            nc.gpsimd.memset(ebias[:], EXP_BIAS)
            nc.gpsimd.dma_start(w2q[:], wq_d[:])
            nc.gpsimd.dma_start(w2k[:], wk_d[:])
            nc.gpsimd.dma_start(w2v[:], wv_d[:])
            nc.gpsimd.dma_start(m2[:], m2_d[:])
            nc.gpsimd.dma_start(idb[:], id_d[:])
            nc.gpsimd.dma_start(relc[:], rel_d[:])
            nc.gpsimd.dma_start(maskx[:], msk_d[:])

            for t in range(NT):
                xt = dp.tile([128, H, W], f32, tag="xt")
                nc.gpsimd.dma_start(xt[0:64, :, :], x_d[2 * t])
                nc.gpsimd.dma_start(xt[64:128, :, :], x_d[2 * t + 1])

                kf = dp.tile([128, HP, HP], f32, tag="kf")
                ve = dp.tile([128, HP, HP], bf16, tag="ve")
                vo = dp.tile([128, HP, HP], bf16, tag="vo")
                qq = dp.tile([128, H, W], f32, tag="qq")
                nc.gpsimd.memset(kf[:], 0.0)
                nc.gpsimd.memset(ve[:], 0.0)
                nc.gpsimd.memset(vo[:], 0.0)

                pq = mp.tile([128, H, W], f32, tag="mm")
                for h in range(2):
                    nc.tensor.matmul(
                        pq[:, 16 * h:16 * h + 16, :], w2q[:],
                        xt[:, 16 * h:16 * h + 16, :], start=True, stop=True)
                nc.scalar.copy(qq[:], pq[:])

                pk = mp.tile([128, H, W], f32, tag="mm")
                for h in range(2):
                    nc.tensor.matmul(
                        pk[:, 16 * h:16 * h + 16, :], w2k[:],
                        xt[:, 16 * h:16 * h + 16, :], start=True, stop=True)
                nc.scalar.copy(kf[:, PAD:PAD + H, PAD:PAD + W], pk[:])

                pv = mp.tile([128, H, W], f32, tag="mm")
                for h in range(2):
                    nc.tensor.matmul(
                        pv[:, 16 * h:16 * h + 16, :], w2v[:],
                        xt[:, 16 * h:16 * h + 16, :], start=True, stop=True)
                nc.scalar.copy(ve[:, PAD:PAD + H, PAD:PAD + W], pv[:])
                nc.scalar.copy(vo[:, PAD:PAD + H, PAD - 1:PAD - 1 + W], pv[:])

                # acc[:, 0] = sum_n u_n ; acc[:, 1] = sum_n e_n   (4 PSUM banks)
                acc = accp.tile([128, 2, H, W], f32, tag="acc")

                for n in range(NSH):
                    ki, kj = divmod(n, KK)
                    tmp = wp.tile([128, H, W], f32, tag="tmp")
                    nc.vector.scalar_tensor_tensor(
                        tmp[:], kf[:, ki:ki + H, kj:kj + W], relc[:, n:n + 1],
                        qq[:], op0=AL.add, op1=AL.mult)
                    sc = mp.tile([128, H, W], f32, tag="mm")
                    for h in range(2):
                        nc.tensor.matmul(
                            sc[:, 16 * h:16 * h + 16, :], m2[:],
                            tmp[:, 16 * h:16 * h + 16, :], start=True, stop=True)
                    ue = wp.tile([128, 2, H, W], bf16, tag="ue")
                    nc.scalar.activation(ue[:, 1], sc[:], AF.Exp, bias=ebias[:])
                    if kj % 2 == 0:
                        vv = ve[:, ki:ki + H, kj:kj + W]
                    else:
                        vv = vo[:, ki:ki + H, kj - 1:kj - 1 + W]
                    nc.vector.tensor_tensor(ue[:, 0], ue[:, 1], vv, op=AL.mult)
                    for j in range(2):
                        for h in range(2):
                            nc.tensor.matmul(
                                acc[:, j, 16 * h:16 * h + 16, :], idb[:],
                                ue[:, j, 16 * h:16 * h + 16, :],
                                start=(n == 0), stop=(n == NSH - 1))

                scl = wp.tile([128, H, W], f32, tag="scl")
                nc.vector.reciprocal_approx_fast(scl[:], acc[:, 1])
                sc2 = wp.tile([128, H, W], f32, tag="sc2")
                nc.vector.tensor_tensor(sc2[:], scl[:], maskx[:], op=AL.mult)
                ot = dp.tile([128, H, W], f32, tag="ot")
                nc.vector.tensor_tensor(ot[:], acc[:, 0], sc2[:], op=AL.mult)
                nc.sync.dma_start(y_d[2 * t], ot[0:64, :, :])
                nc.sync.dma_start(y_d[2 * t + 1], ot[64:128, :, :])

    nc.compile()
    return nc


def _adaptive_mask(current_val):
    template = np.linspace(1.0 - MAXSZ, 0.0, MAXSZ, dtype=np.float64).astype(np.float32)
    om = (template[None, :] + current_val.astype(np.float32) * MAXSZ) / R_RAMP + 1.0
    om = np.clip(om, 0.0, 1.0)                                   # [G, MAXSZ]
    i = np.arange(W)
    r = np.minimum(i, W - 1 - i)
    top = i <= (W - 1 - i)
    lo = np.where(top, r, r + 1)
    hi = W - 1 - r
    c = np.arange(W)
    in_ring = (c[None, :] >= lo[:, None]) & (c[None, :] <= hi[:, None])  # [W,W]
    vals = om[:, r]                                              # [G, W]
    return np.where(in_ring[None, :, :], vals[:, :, None], np.float32(1.0)).astype(np.float32)


def _prep_aux(w_q, w_k, w_v, rel_h, rel_w, current_val):
    def blockdiag(w):
        out = np.zeros((128, 128), np.float32)
        out[:64, :64] = w.T
        out[64:, 64:] = w.T
        return out

    w2q, w2k, w2v = blockdiag(w_q), blockdiag(w_k), blockdiag(w_v)

    p = np.arange(128)
    m = np.arange(128)
    m2 = (((p[:, None] // 64) == (m[None, :] // 64)) &
          (((p[:, None] % 64) // CPG) == ((m[None, :] % 64) // CPG))
          ).astype(np.float32)

    idb = np.eye(128, dtype=ml_dtypes.bfloat16)

    relc = np.zeros((128, NSH), np.float32)
    rh = rel_h.reshape(CO // 2, KK)      # [32, 7] by ki
    rw = rel_w.reshape(CO // 2, KK)      # [32, 7] by kj
    for n in range(NSH):
        ki, kj = divmod(n, KK)
        col = np.concatenate([rh[:, ki], rw[:, kj]])   # [64]
        relc[:64, n] = col
        relc[64:, n] = col

    mask = _adaptive_mask(current_val)                 # [G, W, W]
    cidx = np.arange(64) // CPG
    maskx = mask[cidx]                                 # [64, W, W]
    maskx = np.concatenate([maskx, maskx], axis=0)     # [128, W, W]
    return w2q, w2k, w2v, m2, idb, relc, maskx.astype(np.float32)


def _get_nc():
    if "nc" not in _CACHE:
        _CACHE["nc"] = _build_nc()
    return _CACHE["nc"]


def _run(inputs_list, trace=False):
    from concourse import bass_utils
    nc = _get_nc()
    return bass_utils.run_bass_kernel_spmd(
        nc, inputs_list, core_ids=list(range(NCORES)), trace=trace)


def _make_in_maps(x, w_q, w_k, w_v, rel_h, rel_w, current_val):
    x = np.ascontiguousarray(np.asarray(x, np.float32))
    w2q, w2k, w2v, m2, idb, relc, maskx = _prep_aux(
        np.asarray(w_q, np.float32), np.asarray(w_k, np.float32),
        np.asarray(w_v, np.float32), np.asarray(rel_h, np.float32),
        np.asarray(rel_w, np.float32), np.asarray(current_val, np.float32))
    in_maps = []
    for i in range(NCORES):
        in_maps.append({
            "x": np.ascontiguousarray(x[i * BLOC:(i + 1) * BLOC]),
            "w2q": w2q, "w2k": w2k, "w2v": w2v, "m2": m2, "idb": idb,
            "relc": relc, "maskx": maskx,
        })
    return in_maps


def _assemble(results):
    y = np.concatenate([r["y"] for r in results], axis=0)   # [32, 64, 32, 32]
    return np.ascontiguousarray(y.reshape(B, G, CPG, H, W).astype(np.float32))


def kernel(x, w_q, w_k, w_v, rel_h, rel_w, current_val):
    in_maps = _make_in_maps(x, w_q, w_k, w_v, rel_h, rel_w, current_val)
    res = _run(in_maps, trace=False)
    return _assemble(res.results)


def kernel_traced(x, w_q, w_k, w_v, rel_h, rel_w, current_val):
    """Like kernel(), but also returns the profiled NEFF execution time (ns)."""
    in_maps = _make_in_maps(x, w_q, w_k, w_v, rel_h, rel_w, current_val)
    res = _run(in_maps, trace=True)
    return _assemble(res.results), res.exec_time_ns
